# revision 21
# baseline (speedup 1.0000x reference)
"""Trainium2 Bass kernel for nn_Attention2 (sparse additive attention), v2.

Math (per batch b):
    att_h  = h @ W_h2att.T + b_h2att                       [HID]
    x      = p_att_feats[b] + att_h                        [S, HID]
    scores = tanh(x) @ w_alpha   (+ b_alpha, cancels)      [S]
    scores = where(mask, -1e8, scores)
    w      = softmax(scores)
    out[b] = w @ att_feats[b]                              [RNN]

Data-parallel over batch (16 batches/core on 8 cores). Only unmasked rows
are processed; the host packs them densely (no indirect DMA):
  - "fast" region: each batch's first <=512 unmasked rows occupy exactly 4
    fixed 128-row chunks (padded with p=-sign(w)*15 rows whose softmax
    weight is exp(-sum|w|)~3e-4 and whose A rows are 0).
  - "leftover" region: rows beyond 512 per batch, packed into NLEFT shared
    chunks, batch-bound via host-built one-hot matrices (SPMD-uniform).

Transposed layout for the score phase: p is stored with HID split as
d = c*128 + dp (dp on partitions), so ACT computes tanh(p + att_h) in
ONE pass per (batch, c) using the per-partition bias operand (att_hT
column + folded b_h2att) -- no DVE add, no broadcast matmul.  Scores
are PE matmuls with the tanh tile as the (FWL fp8) stationary operand
and w columns as an N=1 moving operand, accumulated over the 4 d-chunks
in PSUM.  exp on ACT (batched per 8 chunks; 2 for the last groups to
shorten the tail).  Weighted sums are M=16 matmuls with one-hot*e lhsT
(DVE-built; PE psum out base_partition must be 0/32/64, so no M=1 at
partition b) accumulating res [16,1024] + sums in one PSUM region.
b_h2att rides the leftover one-hot as a 17th all-ones row (K=17).

DMA: few big partition-contiguous blocks (128 descriptors each; each
dma_start costs ~1us issue on its queue): weights lead the gpsimd
queue, then interleaved 1MB pT / 2MB A 4-batch blocks; small consts on
sync in parallel.

dtypes: p/A/tanh in fp8 e3m4, W/h/e/one-hots in bf16 (numpy-simulated
rel err 8.7e-3 < 2e-2 tolerance; sim matches HW to 3 digits).
"""

import os
import sys
from contextlib import ExitStack

import numpy as np

for _p in (
    "/root/.axon_site",
    "/root/.axon_site/_ro/trn_rl_repo",
    "/root/.axon_site/_ro/pypackages",
    "/opt/trn_rl_repo",
):
    if os.path.isdir(_p) and _p not in sys.path:
        sys.path.append(_p)

import ml_dtypes
import concourse.bass as bass
import concourse.tile as tile
from concourse import bacc, mybir
from concourse.bass_utils import run_bass_kernel_spmd

B, S, RNN, HID = 128, 1024, 1024, 512
NCORES = 8
BS = B // NCORES          # batches per core
P = 128
NB = 4                    # fast chunks per batch (512 rows)
FAST_ROWS = NB * P        # 512
F32 = mybir.dt.float32
F32R = mybir.dt.float32r
BF16 = mybir.dt.bfloat16
E3 = mybir.dt.float8e3
NP_E3 = ml_dtypes.float8_e3m4
NP_BF16 = ml_dtypes.bfloat16

_PROG_CACHE = {}


def _build_program(nleft: int):
    """nleft: number of leftover 128-row chunks (shared across cores)."""
    key = nleft
    if key in _PROG_CACHE:
        return _PROG_CACHE[key]
    nch = nleft + BS * NB          # total chunks
    rt = nch * P                   # total packed rows
    lrows = nleft * P
    ngroups = (nch + 7) // 8

    nc = bacc.Bacc("TRN2", target_bir_lowering=False, debug=False, num_devices=NCORES)

    pTF_d = nc.dram_tensor("pTF", [P, BS * FAST_ROWS, 4], E3, kind="ExternalInput").ap()
    pTL_d = nc.dram_tensor("pTL", [P, 4, lrows], E3, kind="ExternalInput").ap()
    A_d = nc.dram_tensor("A", [P, nch, RNN], E3, kind="ExternalInput").ap()
    hT_d = nc.dram_tensor("hT", [P, RNN // P, BS], BF16, kind="ExternalInput").ap()
    wT_d = nc.dram_tensor("wT", [RNN, HID], BF16, kind="ExternalInput").ap()
    bhT_d = nc.dram_tensor("bhT", [P, 4], F32, kind="ExternalInput").ap()
    waT_d = nc.dram_tensor("waT", [P, 4], BF16, kind="ExternalInput").ap()
    ones_d = nc.dram_tensor("ones", [P, 1], BF16, kind="ExternalInput").ap()
    ident_d = nc.dram_tensor("ident", [P, P], BF16, kind="ExternalInput").ap()
    ohL_d = nc.dram_tensor("ohL", [BS, lrows], BF16, kind="ExternalInput").ap()
    ohLT_d = nc.dram_tensor("ohLT", [P, nleft, BS], BF16, kind="ExternalInput").ap()
    cm_d = nc.dram_tensor("cm", [P, BS, BS], BF16, kind="ExternalInput").ap()
    out_d = nc.dram_tensor("out", [BS, RNN], F32, kind="ExternalOutput").ap()

    with tile.TileContext(nc) as tc, ExitStack() as ctx:
        res_pool = ctx.enter_context(tc.tile_pool(name="res", bufs=1))
        small = ctx.enter_context(tc.tile_pool(name="small", bufs=4))
        ps_hold = ctx.enter_context(tc.tile_pool(name="ps_hold", bufs=1, space="PSUM"))
        ps_score = ctx.enter_context(tc.tile_pool(name="ps_score", bufs=2, space="PSUM"))
        ps_bc = ctx.enter_context(tc.tile_pool(name="ps_bc", bufs=2, space="PSUM"))
        ps_setup = ctx.enter_context(tc.tile_pool(name="ps_setup", bufs=1, space="PSUM"))

        # ---------- weight loads lead the bulk-data (gpsimd) queue ----------
        wT_sb = res_pool.tile([P, RNN // P, HID], BF16, tag="wT_sb")
        nc.gpsimd.dma_start(out=wT_sb, in_=wT_d.rearrange("(j p) d -> p j d", p=P))
        hT_sb = res_pool.tile([P, RNN // P, BS], BF16, tag="hT_sb")
        nc.gpsimd.dma_start(out=hT_sb, in_=hT_d)
        # ---------- small constant loads (sync queue, parallel) ----------
        bhT_sb = res_pool.tile([P, 4], F32, tag="bhT_sb")
        nc.sync.dma_start(out=bhT_sb, in_=bhT_d)
        waT_sb = res_pool.tile([P, 4], BF16, tag="waT_sb")
        nc.sync.dma_start(out=waT_sb, in_=waT_d)
        ones_sb = res_pool.tile([P, 1], BF16, tag="ones_sb")
        nc.sync.dma_start(out=ones_sb, in_=ones_d)
        ident_sb = res_pool.tile([P, P], BF16, tag="ident_sb")
        nc.sync.dma_start(out=ident_sb, in_=ident_d)
        ohL_sb = res_pool.tile([BS, lrows], BF16, tag="ohL_sb")
        nc.sync.dma_start(out=ohL_sb, in_=ohL_d)
        ohLT_sb = res_pool.tile([P, nleft, BS], BF16, tag="ohLT_sb")
        nc.sync.dma_start(out=ohLT_sb, in_=ohLT_d)
        cm_sb = res_pool.tile([P, BS, BS], BF16, tag="cm_sb")
        nc.sync.dma_start(out=cm_sb, in_=cm_d)

        # ---------- bulk data loads: 4-batch blocks (128 descs each) ----------
        pTL = res_pool.tile([P, 4, lrows], E3, tag="pTL")
        nc.gpsimd.dma_start(out=pTL, in_=pTL_d)
        AL = res_pool.tile([P, nleft, RNN], E3, tag="AL")
        nc.gpsimd.dma_start(out=AL, in_=A_d[:, 0:nleft, :])
        BB = 4 * FAST_ROWS
        pTF = []
        AF = []
        for i in range(BS // 4):
            t = res_pool.tile([P, BB, 4], E3, tag=f"pTF{i}", name=f"pTF{i}")
            nc.gpsimd.dma_start(out=t, in_=pTF_d[:, i * BB : (i + 1) * BB, :])
            pTF.append(t)
            c0 = nleft + i * 4 * NB
            a = res_pool.tile([P, 4 * NB, RNN], E3, tag=f"AF{i}", name=f"AF{i}")
            nc.gpsimd.dma_start(out=a, in_=A_d[:, c0 : c0 + 4 * NB, :])
            AF.append(a)

        # ---------- setup: att_hT[dp, c, b] = sum_k W.T[k, c*128+dp] h[b, k] + bh
        # computed directly in transposed form (wT chunk stationary, h moving)
        attT_ps = ps_setup.tile([P, 4, BS], F32, tag="attT_ps")
        for c in range(4):
            for j in range(RNN // P):
                nc.tensor.matmul(
                    out=attT_ps[:, c, :],
                    lhsT=wT_sb[:, j, c * P : (c + 1) * P],
                    rhs=hT_sb[:, j, :],
                    start=(j == 0),
                    stop=(j == RNN // P - 1),
                )
        att_hT_sb = res_pool.tile([P, 4, BS], BF16, tag="att_hT_sb")
        nc.vector.tensor_add(
            att_hT_sb, attT_ps, bhT_sb.unsqueeze(2).broadcast_to([P, 4, BS])
        )

        # row-layout att_h (leftover path only; off the critical path)
        atthp = ps_setup.tile([BS, 4, P], BF16, tag="atthp")
        for c in range(4):
            nc.tensor.transpose(
                out=atthp[:, c, :], in_=att_hT_sb[:, c, :], identity=ident_sb
            )
        att_h_sb = res_pool.tile([BS, 4, P], BF16, tag="att_h_sb")
        nc.vector.tensor_copy(att_h_sb, atthp)

        # tanh storage: leftover + one tile per batch
        tanhL = res_pool.tile([P, 4, lrows], E3, tag="tanhL")
        tanhF = [res_pool.tile([P, 4, FAST_ROWS], E3, tag=f"tF{b}", name=f"tF{b}") for b in range(BS)]
        e_sb = res_pool.tile([P, ngroups * 8], BF16, tag="e_sb")

        hold = ps_hold.tile([P, RNN], F32)
        res_ps = hold[0:BS, :]
        sums_ps = hold[64 : 64 + BS, 0:1]

        # ---------- helpers ----------
        def tanh_ap(k):
            """(lhsT source) tanh tile + row slice for chunk k, d-chunk c."""
            if k < nleft:
                return tanhL, k * P
            kf = k - nleft
            return tanhF[kf // NB], (kf % NB) * P

        def emit_leftover_pre(l):
            # bc = one-hot broadcast of att_h to leftover rows (transposed)
            bc = ps_bc.tile([P, 4, P], F32, tag="bc")
            for c in range(4):
                nc.tensor.matmul(
                    out=bc[:, c, :],
                    lhsT=att_h_sb[:, c, :],
                    rhs=ohL_sb[:, l * P : (l + 1) * P],
                    start=True,
                    stop=True,
                )  # att_h_sb already includes b_h2att
            x = small.tile([P, 4, P], BF16, tag="xL")
            nc.vector.tensor_add(x, bc, pTL[:, :, l * P : (l + 1) * P])
            nc.scalar.activation(
                out=tanhL[:, :, l * P : (l + 1) * P],
                in_=x,
                func=mybir.ActivationFunctionType.Tanh,
            )

        def emit_fast_tanh(b):
            blk, r0 = b // 4, (b % 4) * FAST_ROWS
            for c in range(4):
                nc.scalar.activation(
                    out=tanhF[b][:, c, :],
                    in_=pTF[blk][:, r0 : r0 + FAST_ROWS, c],
                    func=mybir.ActivationFunctionType.Tanh,
                    bias=att_hT_sb[:, c, b : b + 1],
                )

        def emit_score(k, sc_tile, slot):
            t, r0 = tanh_ap(k)
            for c in range(4):
                nc.tensor.matmul(
                    out=sc_tile[:, slot : slot + 1],
                    lhsT=t[:, c, r0 : r0 + P],
                    rhs=waT_sb[:, c : c + 1],
                    start=(c == 0),
                    stop=(c == 3),
                )

        def emit_res(k):
            st = k == 0
            sp = k == nch - 1
            if k < nleft:
                oh_src = ohLT_sb[:, k, :]
                rhs_t, j = AL, k
            else:
                kf = k - nleft
                b = kf // NB
                oh_src = cm_sb[:, b, :]
                rhs_t, j = AF[kf // (4 * NB)], kf % (4 * NB)
            ohw = small.tile([P, BS], BF16, tag="ohw")
            nc.vector.tensor_mul(
                ohw, oh_src, e_sb[:, k : k + 1].broadcast_to([P, BS])
            )
            if sp:
                nc.tensor.matmul(out=sums_ps, lhsT=ohw, rhs=ones_sb, start=st, stop=sp)
            nc.tensor.matmul(
                out=res_ps[:, 0:512], lhsT=ohw, rhs=rhs_t[:, j, 0:512], start=st, stop=sp
            )
            nc.tensor.matmul(
                out=res_ps[:, 512:1024], lhsT=ohw, rhs=rhs_t[:, j, 512:1024],
                start=st, stop=sp,
            )
            if not sp:
                nc.tensor.matmul(out=sums_ps, lhsT=ohw, rhs=ones_sb, start=st, stop=sp)

        # ---------- main pipeline ----------
        tanh_done = set()
        for g in range(ngroups):
            ks = [k for k in range(8 * g, min(8 * g + 8, nch))]
            sc = ps_score.tile([P, 8], F32, tag="score")
            for k in sorted(ks, key=lambda k: (k < nleft, k)):
                if k < nleft:
                    emit_leftover_pre(k)
                else:
                    b = (k - nleft) // NB
                    if b not in tanh_done:
                        tanh_done.add(b)
                        emit_fast_tanh(b)
                emit_score(k, sc, k - 8 * g)
            sub = 2 if g >= ngroups - 4 else 8
            for i0 in range(0, len(ks), sub):
                kk = ks[i0 : i0 + sub]
                nc.scalar.activation(
                    out=e_sb[:, kk[0] : kk[0] + len(kk)],
                    in_=sc[:, i0 : i0 + len(kk)],
                    func=mybir.ActivationFunctionType.Exp,
                )
                for k in kk:
                    emit_res(k)

        # ---------- normalize + store ----------
        recip_sb = res_pool.tile([BS, 1], F32, tag="recip_sb")
        nc.vector.reciprocal(recip_sb, sums_ps)
        out_sb = res_pool.tile([BS, RNN], F32, tag="out_sb")
        for hh in range(2):
            sl = slice(hh * 512, (hh + 1) * 512)
            nc.vector.tensor_scalar_mul(
                out=out_sb[:, sl], in0=res_ps[:, sl], scalar1=recip_sb
            )
            nc.sync.dma_start(out=out_d[:, sl], in_=out_sb[:, sl])

    nc.compile()
    _PROG_CACHE[key] = nc
    return nc


def _pack_core(m, p_flat, A_flat, mask, wa):
    """Host-side packing for core m. Returns (rows_fast[BS,512], left_rows, left_b)."""
    mask_m = mask[m * BS : (m + 1) * BS]
    fast = np.full((BS, FAST_ROWS), -1, np.int64)
    left_rows = []
    left_b = []
    for b in range(BS):
        idx = np.flatnonzero(~mask_m[b])
        n = min(len(idx), FAST_ROWS)
        fast[b, :n] = b * S + idx[:n]
        if len(idx) > FAST_ROWS:
            extra = b * S + idx[FAST_ROWS:]
            left_rows.append(extra)
            left_b.append(np.full(len(extra), b, np.int64))
    left_rows = np.concatenate(left_rows) if left_rows else np.empty(0, np.int64)
    left_b = np.concatenate(left_b) if left_b else np.empty(0, np.int64)
    return fast, left_rows, left_b


def run(inputs, trace: bool = False, trace_kwargs: dict | None = None, **_ignored):
    h = np.asarray(inputs["h"], dtype=np.float32)
    A = np.asarray(inputs["att_feats"], dtype=np.float32)
    p = np.asarray(inputs["p_att_feats"], dtype=np.float32)
    mask = np.asarray(inputs["mask"]).astype(bool)
    W = np.asarray(inputs["W_h2att"], dtype=np.float32)
    bh = np.asarray(inputs["b_h2att"], dtype=np.float32)
    wa = np.asarray(inputs["w_alpha"], dtype=np.float32)

    packs = [
        _pack_core(m, None, None, mask, wa) for m in range(NCORES)
    ]
    nleft = max(1, max((len(lr) + P - 1) // P for (_, lr, _) in packs))
    nch = nleft + BS * NB
    rt = nch * P
    lrows = nleft * P

    pad_row = (-np.sign(wa) * 15.0).astype(np.float32)
    pad_row[wa == 0] = -15.0

    shared = {
        "wT": np.ascontiguousarray(W.T).astype(NP_BF16),
        "bhT": np.ascontiguousarray(bh.reshape(4, P).T),
        "waT": np.ascontiguousarray(wa.reshape(4, P).T).astype(NP_BF16),
        "ones": np.ones((P, 1), NP_BF16),
        "ident": np.eye(P, dtype=np.float32).astype(NP_BF16),
        "cm": np.ascontiguousarray(
            np.broadcast_to(np.eye(BS, dtype=np.float32), (P, BS, BS)).transpose(0, 2, 1)
        ).astype(NP_BF16),
    }

    in_maps = []
    for m in range(NCORES):
        fast, left_rows, left_b = packs[m]
        sl = slice(m * BS, (m + 1) * BS)
        p_m = p[sl].reshape(BS * S, HID)
        A_m = A[sl].reshape(BS * S, RNN)

        rows = np.full(rt, -1, np.int64)
        rows[:len(left_rows)] = left_rows
        rows[lrows:] = fast.reshape(-1)
        valid = rows >= 0

        pg = np.empty((rt, HID), np.float32)
        pg[valid] = p_m[rows[valid]]
        pg[~valid] = pad_row
        pg3 = pg.reshape(rt, 4, P).astype(NP_E3)
        pTL_host = np.ascontiguousarray(pg3[:lrows].transpose(2, 1, 0))
        pTF_host = np.ascontiguousarray(pg3[lrows:].transpose(2, 0, 1))

        Ag = np.zeros((rt, RNN), np.float32)
        Ag[valid] = A_m[rows[valid]]
        A_host = np.ascontiguousarray(
            Ag.reshape(nch, P, RNN).transpose(1, 0, 2)
        ).astype(NP_E3)

        ohL = np.zeros((BS, lrows), np.float32)
        ohL[left_b, np.arange(len(left_b))] = 1.0
        ohLT = np.ascontiguousarray(
            ohL.T.reshape(nleft, P, BS).transpose(1, 0, 2)
        ).astype(NP_BF16)

        in_map = dict(shared)
        in_map.update(
            {
                "pTF": pTF_host,
                "pTL": pTL_host,
                "A": A_host,
                "hT": np.ascontiguousarray(
                    h[sl].T.reshape(RNN // P, P, BS).transpose(1, 0, 2)
                ).astype(NP_BF16),
                "ohL": ohL.astype(NP_BF16),
                "ohLT": ohLT,
            }
        )
        in_maps.append(in_map)

    nc = _build_program(nleft)
    br = run_bass_kernel_spmd(
        nc,
        in_maps,
        core_ids=list(range(NCORES)),
        trace=trace,
        **(trace_kwargs or {}),
    )
    out = np.concatenate([br.results[m]["out"] for m in range(NCORES)], axis=0)
    out = out.astype(np.float32)

    # Fully-masked batches: reference softmax degenerates to uniform weights.
    dead = np.flatnonzero(mask.all(axis=1))
    for b in dead:
        out[b] = A[b].mean(axis=0, dtype=np.float64).astype(np.float32)
    return out, br


def kernel(**inputs) -> np.ndarray:
    out, _ = run(inputs, trace=False)
    return out
